# revision 1
# baseline (speedup 1.0000x reference)
"""3-layer GCN (message passing) on 8 Trainium2 NeuronCores, SPMD.

Strategy (graph/data parallel, nodes sharded by id across 8 cores):
  - Nodes sharded by id (25K/core + padding); each core owns all in-edges of
    its nodes (dst-sharded edges).
  - Per layer, each core builds its shard of the scaled node table
    (h*norm*mask)@W as 256B rows, AllGathers the table, then aggregates its
    edges in 8 per-source-owner passes: Ant dma_gather (int16 local indices
    into that owner's 32K-row table slice) -> DVE halving-tree window reduce
    (exact sub-degree classes) -> Ant dma_scatter_add of per-node partials
    into a DRAM accumulator keyed by the node's own table row.
  - z = norm*(agg + self_row); global BatchNorm stats via ones-matmuls +
    AllReduce; h = relu(z*scale+shift). GCN biases drop out under BN.
  - y = h3 @ fcW + fcb on DVE; host unshards/unpermutes.
"""

import os
import numpy as np

N_NODES = 200000
N_EDGES = 2500000
H = 32
EL = 64           # gather/scatter element = 64 fp32 = 256B
NC = 8
EPS = 1e-5

GCHUNK = 4096     # idxs per dma_gather
SCHUNK = 4096     # idxs per dma_scatter_add
MERGE_MIN = 256   # min per-(core,owner) class size before merging degrees


def _wrap16(a):
    """flat int array [n] -> [128, n//16] wrapped-16, replicated x8."""
    n = len(a)
    assert n % 16 == 0
    blk = a.reshape(n // 16, 16).T  # [16, n/16]
    return np.tile(blk, (8, 1)).astype(np.int16)


# ---------------------------------------------------------------------------
# Host-side planning
# ---------------------------------------------------------------------------

def build_plan(src, dst, x, n_nodes=N_NODES, nc_cores=NC):
    nper = n_nodes // nc_cores
    deg = np.bincount(dst, minlength=n_nodes).astype(np.int64)
    core_of = (np.arange(n_nodes) // nper).astype(np.int64)
    src_owner = (src // nper).astype(np.int64)

    # (dst, owner) sub-degrees
    key = dst * nc_cores + src_owner
    subdeg = np.bincount(key, minlength=n_nodes * nc_cores)\
        .reshape(n_nodes, nc_cores)

    # edges sorted by (dst, owner) for segment extraction
    order = np.lexsort((src_owner, dst))
    src_sorted = src[order].astype(np.int64)
    seg_starts = np.zeros(n_nodes * nc_cores + 1, np.int64)
    seg_starts[1:] = np.cumsum(subdeg.reshape(-1))

    # global node rank: group nodes per core by total degree class (any
    # grouping works; degree-grouping keeps things balanced)
    edeg = np.maximum(deg, 1)
    maxd = int(edeg.max())
    hist = np.zeros((nc_cores, maxd + 1), np.int64)
    np.add.at(hist, (core_of, edeg), 1)
    gclasses = []
    run = []
    for d in range(1, maxd + 1):
        if hist[:, d].sum() == 0 and not run:
            continue
        run.append(d)
        percore = hist[:, run].sum(axis=1)
        if percore.max() >= 2048 or d == maxd:
            cap = int(max(128, -(-int(percore.max()) // 128) * 128))
            gclasses.append({"dlist": list(run), "cap": cap})
            run = []
    total_cap = sum(c["cap"] for c in gclasses)
    if max(int(hist[c, 1:].sum()) for c in range(nc_cores)) >= total_cap:
        gclasses[0]["cap"] += 128
    npad = sum(c["cap"] for c in gclasses)
    nf = npad // 128
    assert npad < 32768, npad

    cls_of_deg = np.zeros(maxd + 1, np.int64)
    for ci, c in enumerate(gclasses):
        for d in c["dlist"]:
            cls_of_deg[d] = ci
    gcls = cls_of_deg[edeg]
    rank_of = np.full(n_nodes, -1, np.int64)
    off = 0
    for ci, c in enumerate(gclasses):
        for cc in range(nc_cores):
            sel = np.nonzero((core_of == cc) & (gcls == ci))[0]
            rank_of[sel] = off + np.arange(len(sel))
        off += c["cap"]
    assert (rank_of >= 0).all()
    p_of = rank_of % 128
    g_of = rank_of // 128
    trow_of = p_of * nf + g_of          # local table row within owner shard

    # dummy (zero) table row per core
    dummy_row = np.zeros(nc_cores, np.int64)
    for cc in range(nc_cores):
        used = np.zeros(npad, bool)
        used[rank_of[core_of == cc]] = True
        free = np.nonzero(~used)[0]
        assert len(free) > 0
        r = free[0]
        dummy_row[cc] = (r % 128) * nf + (r // 128)

    # ---- per-owner window structures ----
    # per (owner o): classes over subdeg_o >= 1 (exact; tail merged)
    smax = int(subdeg.max())
    # per (core, owner, d) histogram
    shist = np.zeros((nc_cores, nc_cores, smax + 1), np.int64)
    np.add.at(shist, (core_of[:, None].repeat(nc_cores, 1),
                      np.arange(nc_cores)[None, :].repeat(n_nodes, 0),
                      subdeg), 1)

    owners = []
    gidx_parts = []   # per-core list of flat int16 gather idx streams
    sidx_parts = []
    gidx_flat = [[] for _ in range(nc_cores)]
    sidx_flat = [[] for _ in range(nc_cores)]

    for o in range(nc_cores):
        # class list for this owner
        oclasses = []
        run = []
        for d in range(1, smax + 1):
            if shist[:, o, d].sum() == 0 and not run:
                continue
            run.append(d)
            percore = shist[:, o, run].sum(axis=1)
            if percore.max() >= MERGE_MIN or d == smax:
                if percore.max() > 0:
                    cap = int(max(128, -(-int(percore.max()) // 128) * 128))
                    oclasses.append({"dlist": list(run), "delta": run[-1],
                                     "cap": cap})
                run = []
        ocls_of_d = np.full(smax + 1, -1, np.int64)
        for ci, c in enumerate(oclasses):
            assert c["delta"] <= GCHUNK // 128, c
            for d in c["dlist"]:
                ocls_of_d[d] = ci

        # slot columns + compact columns
        slot_cols = sum(c["cap"] // 128 * c["delta"] for c in oclasses)
        comp_cols = sum(c["cap"] // 128 for c in oclasses)

        frags = []   # (slot_col_off, ng, delta, comp_col_off)
        scol = 0
        ccol = 0
        for c in oclasses:
            ng = c["cap"] // 128
            frags.append((scol, ng, c["delta"], ccol))
            scol += ng * c["delta"]
            ccol += ng

        owners.append({"classes": oclasses, "frags": frags,
                       "slot_cols": slot_cols, "comp_cols": comp_cols})

        # per-core index streams
        osub = subdeg[:, o]
        onc = ocls_of_d[np.minimum(osub, smax)]
        for cc in range(nc_cores):
            gstream = np.full(slot_cols * 128, dummy_row[o], np.int64)
            sstream = np.full(comp_cols * 128, dummy_row[cc], np.int64)
            scol = 0
            ccol = 0
            for ci, c in enumerate(oclasses):
                delta, cap = c["delta"], c["cap"]
                ng = cap // 128
                sel = np.nonzero((core_of == cc) & (osub >= 1)
                                 & (onc == ci))[0]
                sel = sel[np.argsort(rank_of[sel], kind="stable")]
                nsel = len(sel)
                assert nsel <= cap
                # gather idx: member i -> partition i%128, cols
                # [scol + (i//128)*delta + j]; stream index k = p + 128*s
                S = np.full((cap, delta), dummy_row[o], np.int64)
                if nsel:
                    st = seg_starts[sel * nc_cores + o]
                    dg = osub[sel]
                    for j in range(delta):
                        m = dg > j
                        if m.any():
                            S[np.nonzero(m)[0], j] = \
                                trow_of[src_sorted[st[m] + j]]
                # member i = (p, g): columns (g*delta + j); value at
                # stream pos k = p + 128*(scol + g*delta + j)
                Sv = S.reshape(ng, 128, delta)        # [g, p, j]
                block = np.transpose(Sv, (0, 2, 1))   # [g, j, p]
                gstream[scol * 128:(scol + ng * delta) * 128] = \
                    block.reshape(-1)
                # scatter idx: member i -> own table row; dummies -> own
                # core dummy row (adds zeros)
                C = np.full((cap,), dummy_row[cc], np.int64)
                if nsel:
                    C[:nsel] = trow_of[sel]
                Cv = C.reshape(ng, 128)
                sstream[ccol * 128:(ccol + ng) * 128] = Cv.reshape(-1)
                scol += ng * delta
                ccol += ng
            gidx_flat[cc].append(gstream)
            sidx_flat[cc].append(sstream)

    # concat per-core streams; record per-owner offsets (in idx units)
    goff = [0]
    soff = [0]
    for o in range(nc_cores):
        goff.append(goff[-1] + owners[o]["slot_cols"] * 128)
        soff.append(soff[-1] + owners[o]["comp_cols"] * 128)
    gidx_all = np.stack([np.concatenate(gidx_flat[cc]) for cc in
                         range(nc_cores)])
    sidx_all = np.stack([np.concatenate(sidx_flat[cc]) for cc in
                         range(nc_cores)])
    gidx_w = np.stack([_wrap16(gidx_all[cc]) for cc in range(nc_cores)])
    sidx_w = np.stack([_wrap16(sidx_all[cc]) for cc in range(nc_cores)])

    def to_tile(vals_full, cc):
        arr = np.zeros(npad, np.float32)
        sel = core_of == cc
        arr[rank_of[sel]] = vals_full[sel]
        return arr.reshape(nf, 128).T.copy()

    x_t = np.stack([to_tile(np.asarray(x, np.float32).reshape(-1), cc)
                    for cc in range(nc_cores)])
    deg_t = np.stack([to_tile(deg.astype(np.float32), cc)
                      for cc in range(nc_cores)])
    mask_t = np.stack([to_tile(np.ones(n_nodes, np.float32), cc)
                       for cc in range(nc_cores)])

    tot_slots = goff[-1]
    return {
        "nper": nper, "npad": npad, "nf": nf,
        "owners": owners, "goff": goff, "soff": soff,
        "tot_slots": tot_slots, "tot_comp": soff[-1],
        "x_t": x_t, "deg_t": deg_t, "mask_t": mask_t,
        "gidx_w": gidx_w, "sidx_w": sidx_w,
        "rank_of": rank_of, "core_of": core_of, "n_nodes": n_nodes,
        "nc_cores": nc_cores,
    }


# ---------------------------------------------------------------------------
# Device program
# ---------------------------------------------------------------------------

def _patch_queue_aware_lanes():
    """Make Tile's DMASW lane assignment queue-consistent: lane =
    queue_num*2 + toggle. Without this, multi-queue SWDGE programs get
    lanes shared across queues (sim rejects; HW would race)."""
    import concourse.tile_sem_assignment as tsa
    import concourse.mybir as mybir
    if getattr(tsa, "_gnn_qpatch", False):
        return
    cls = None
    for name in dir(tsa):
        obj = getattr(tsa, name)
        if isinstance(obj, type) and hasattr(obj, "_assign_tick"):
            cls = obj
            break
    assert cls is not None, "no _assign_tick owner found"
    orig = cls._assign_tick

    def patched(self, inst):
        qn = getattr(inst, "queue_num", None)
        if (qn is not None and inst.engine == mybir.EngineType.Pool
                and self.swdge_sem_count >= 8):
            if not hasattr(self, "_gnn_qtog"):
                self._gnn_qtog = {}
            tog = self._gnn_qtog.get(qn, 0)
            self._gnn_qtog[qn] = tog ^ 1
            lane = (qn * 2 + tog) % self.swdge_sem_count
            save = self.next_sw_dma_idx
            self.next_sw_dma_idx = lane
            try:
                return orig(self, inst)
            finally:
                self.next_sw_dma_idx = save
        return orig(self, inst)

    cls._assign_tick = patched
    tsa._gnn_qpatch = True


def build_program(plan):
    import contextlib
    import concourse.bacc as bacc
    import concourse.bass as bass
    import concourse.mybir as mybir
    import concourse.tile as tile
    from concourse.replica_groups import maybe_share_collective_output_space
    if int(os.environ.get("GNN_NQ", "4")) > 1:
        _patch_queue_aware_lanes()

    f32 = mybir.dt.float32
    i16 = mybir.dt.int16
    ADD = mybir.AluOpType.add
    SUB = mybir.AluOpType.subtract
    MULT = mybir.AluOpType.mult

    nf = plan["nf"]
    npad = plan["npad"]
    ncc = plan["nc_cores"]
    n_real = float(plan["n_nodes"])
    groups = [list(range(ncc))]
    owners = plan["owners"]
    goff = plan["goff"]
    soff = plan["soff"]

    nc = bacc.Bacc("TRN2", target_bir_lowering=False, debug=False,
                   num_devices=ncc, num_swdge_queues=4)

    # ---- I/O ----
    xin = nc.dram_tensor("xin", [128, nf], f32, kind="ExternalInput").ap()
    degin = nc.dram_tensor("degin", [128, nf], f32, kind="ExternalInput").ap()
    maskin = nc.dram_tensor("maskin", [128, nf], f32,
                            kind="ExternalInput").ap()
    gidxin = nc.dram_tensor("gidxin", [128, plan["tot_slots"] // 16], i16,
                            kind="ExternalInput").ap()
    sidxin = nc.dram_tensor("sidxin", [128, plan["tot_comp"] // 16], i16,
                            kind="ExternalInput").ap()
    win = {}
    for name, shp in [("w1", [1, H]), ("w2", [H, H]), ("w3", [H, H]),
                      ("fcw", [1, H]), ("fcb", [1, 1]),
                      ("g1", [1, H]), ("be1", [1, H]),
                      ("g2", [1, H]), ("be2", [1, H]),
                      ("g3", [1, H]), ("be3", [1, H])]:
        win[name] = nc.dram_tensor(name, shp, f32, kind="ExternalInput").ap()
    yout = nc.dram_tensor("yout", [128, nf], f32, kind="ExternalOutput").ap()

    # ---- internal DRAM ----
    shared = maybe_share_collective_output_space("AllGather", groups)
    tsh = nc.dram_tensor("tsh", [npad, EL], f32, kind="Internal")
    tall = [nc.dram_tensor(f"tall{i}", [ncc * npad, EL], f32,
                           kind="Internal", addr_space=shared)
            for i in range(3)]
    aggD = [nc.dram_tensor(f"aggD{i}", [npad, EL], f32, kind="Internal")
            for i in range(3)]
    stb_in = [nc.dram_tensor(f"stin{i}", [1, 2 * H], f32, kind="Internal")
              for i in range(3)]
    stb_out = [nc.dram_tensor(f"stout{i}", [1, 2 * H], f32, kind="Internal")
               for i in range(3)]

    def ap_append(ap, dims):
        return bass.AP(ap.tensor, ap.offset, list(ap.ap) + list(dims))

    def bc_feat(ap2d, w=H):
        return ap_append(ap2d, [[0, w]])

    def row_bc(ap_row):
        a = list(ap_row.ap)
        return bass.AP(ap_row.tensor, ap_row.offset, [a[0], [0, nf]] + a[1:])

    nq = int(os.environ.get("GNN_NQ", "4"))
    qn = [0]

    def next_q():
        if nq == 1:
            return 0
        qn[0] = qn[0] % (nq - 1) + 1   # rotate 1..nq-1 (gathers)
        return qn[0]

    with tile.TileContext(nc) as tc:
        with contextlib.ExitStack() as ctx:
            sb = ctx.enter_context(tc.tile_pool(name="sb", bufs=1))
            msgp = ctx.enter_context(tc.tile_pool(name="msg", bufs=3))
            idxp = ctx.enter_context(tc.tile_pool(name="idxp", bufs=3))
            compp = ctx.enter_context(tc.tile_pool(name="compp", bufs=2))
            smp = ctx.enter_context(tc.tile_pool(name="small", bufs=1))
            bcp = ctx.enter_context(tc.tile_pool(name="bc", bufs=2))
            psp = ctx.enter_context(
                tc.tile_pool(name="ps", bufs=2, space="PSUM"))
            pstat = ctx.enter_context(
                tc.tile_pool(name="pstat", bufs=1, space="PSUM"))

            xs = smp.tile([128, nf], f32, tag="xs")
            nc.sync.dma_start(xs[:], xin)
            degs = smp.tile([128, nf], f32, tag="degs")
            nc.sync.dma_start(degs[:], degin)
            masks = smp.tile([128, nf], f32, tag="masks")
            nc.sync.dma_start(masks[:], maskin)

            wt = {}
            for name in win:
                shp = list(win[name].shape)
                wt[name] = smp.tile(shp, f32, tag=f"wt_{name}",
                                    name=f"wt_{name}")
                nc.sync.dma_start(wt[name][:], win[name])

            c0 = smp.tile([128, 1], f32, tag="c0")
            nc.gpsimd.memset(c0[:], 0.0)
            nc.const_aps.aps[(f32, 0.0)] = c0[:]
            ceps = smp.tile([128, 1], f32, tag="ceps")
            nc.gpsimd.memset(ceps[:], EPS)
            nc.const_aps.aps[(f32, EPS)] = ceps[:]

            ident = smp.tile([128, 128], f32, tag="ident")
            from concourse.masks import make_identity
            make_identity(nc, ident[:])
            ones_row = smp.tile([1, 128], f32, tag="ones_row")
            nc.gpsimd.memset(ones_row[:], 1.0)
            ones_col = smp.tile([128, 1], f32, tag="ones_col")
            nc.gpsimd.memset(ones_col[:], 1.0)

            # norm = rsqrt(deg + 1); nm = norm * mask
            norm = smp.tile([128, nf], f32, tag="norm")
            nc.vector.tensor_scalar_add(norm[:], degs[:], 1.0)
            nc.vector.reciprocal(norm[:], norm[:])
            nc.scalar.sqrt(norm[:], norm[:])
            nm = smp.tile([128, nf], f32, tag="nm")
            nc.vector.tensor_tensor(nm[:], norm[:], masks[:], op=MULT)

            # big state tiles
            hT = sb.tile([128, nf, H], f32, tag="h")
            zT = sb.tile([128, nf, H], f32, tag="z")
            stag = sb.tile([128, nf, EL], f32, tag="stag")  # table staging
            nc.gpsimd.memset(stag[:], 0.0)   # cols H..EL stay zero forever
            zero64 = None

            def pe_broadcast_row(row_ap, width):
                ps = psp.tile([128, width], f32, tag="ps_bc")
                nc.tensor.matmul(ps[:], ones_row[:], row_ap,
                                 start=True, stop=True)
                out = bcp.tile([128, width], f32, tag="sb_bc")
                nc.vector.tensor_copy(out[:], ps[:])
                return out

            def emit_aggregate(tall_h, aggd, li, width):
                """Per-owner gather+tree+scatter into aggd; then read agg."""
                # zero the accumulator (broadcast small zero tile)
                import concourse.bass as bassz
                zsrc = bassz.AP(zero64.tensor, zero64[:].offset,
                                [zero64[:].ap[0], [0, nf], [1, EL]])
                nc.sync.dma_start(
                    aggd.ap().rearrange("(p g) f -> p g f", p=128), zsrc)
                for o in range(ncc):
                    ow = owners[o]
                    in_view = tall_h.ap()[o * npad:(o + 1) * npad, :]
                    cols_per_chunk = GCHUNK // 128
                    frags = ow["frags"]

                    def aligned_take(col, want, ow=ow, frags=frags):
                        cut = min(col + want, ow["slot_cols"])
                        for (scol, ng, delta, ccol) in frags:
                            if scol < cut < scol + ng * delta:
                                cut -= (cut - scol) % delta
                                break
                        assert cut > col, "window wider than chunk"
                        return cut - col

                    col = 0
                    while col < ow["slot_cols"]:
                        take = aligned_take(col, cols_per_chunk)
                        nidx = take * 128
                        mt = msgp.tile([128, cols_per_chunk, EL], f32,
                                       tag="msg")
                        it = idxp.tile([128, GCHUNK // 16], i16, tag="gix")
                        base = goff[o] + col * 128
                        nc.sync.dma_start(
                            it[:, :nidx // 16],
                            gidxin[:, base // 16:(base + nidx) // 16])
                        nc.gpsimd.dma_gather(
                            out_ap=mt[:, :take, :], in_ap=in_view,
                            idxs_ap=it[:, :nidx // 16],
                            num_idxs=nidx, num_idxs_reg=nidx,
                            elem_size=EL, single_packet=False,
                            queue_num=next_q())
                        comp = compp.tile([128, cols_per_chunk, EL], f32,
                                          tag="comp")
                        cc_lo = None
                        cc_hi = None
                        for (scol, ng, delta, ccol) in frags:
                            lo = max(scol, col)
                            hi = min(scol + ng * delta, col + take)
                            if lo >= hi:
                                continue
                            assert (lo - scol) % delta == 0
                            assert (hi - scol) % delta == 0
                            g0 = (lo - scol) // delta
                            g1 = (hi - scol) // delta
                            a = lo - col
                            ngf = g1 - g0
                            V = mt[:, a:a + ngf * delta, :].rearrange(
                                "p (g j) f -> p g j f", j=delta)
                            t = delta
                            while t > 1:
                                hh = t - t // 2
                                k = t // 2
                                nc.vector.tensor_tensor(
                                    V[:, :, 0:k, 0:width],
                                    V[:, :, 0:k, 0:width],
                                    V[:, :, hh:hh + k, 0:width], op=ADD)
                                t = hh
                            Vflat = mt[:, a:a + ngf * delta, :].rearrange(
                                "p (g j) f -> p g (j f)", j=delta)
                            if cc_lo is None:
                                cc_lo = ccol + g0
                            ccur = ccol + g0
                            nc.vector.tensor_copy(
                                comp[:, ccur - cc_lo:ccur - cc_lo + ngf, :],
                                Vflat[:, :, 0:EL])
                            cc_hi = ccol + g1
                        assert cc_lo is not None
                        ncomp = (cc_hi - cc_lo) * 128
                        sit = idxp.tile([128, GCHUNK // 16], i16, tag="six")
                        sbase = soff[o] + cc_lo * 128
                        nc.sync.dma_start(
                            sit[:, :ncomp // 16],
                            sidxin[:, sbase // 16:(sbase + ncomp) // 16])
                        nc.gpsimd.dma_scatter_add(
                            out_ap=aggd.ap(),
                            in_ap=comp[:, :cc_hi - cc_lo, :],
                            idxs_ap=sit[:, :ncomp // 16],
                            num_idxs=ncomp, num_idxs_reg=ncomp,
                            elem_size=EL, single_packet=False,
                            queue_num=0)
                        col += take
                # read back agg (useful cols only)
                nc.sync.dma_start(
                    zT[:] if width == H else zT[:, :, 0:1],
                    aggd.ap().rearrange("(p g) f -> p g f", p=128)
                    [:, :, 0:width])

            def emit_stats(z_tile, zsq_tile, li):
                nc.vector.tensor_tensor(zsq_tile[:], z_tile[:], z_tile[:],
                                        op=MULT)
                pss = pstat.tile([1, 16 * H], f32, tag="pss")
                psq = pstat.tile([1, 16 * H], f32, tag="psq")
                nblk = (nf + 15) // 16
                for b in range(nblk):
                    g0 = b * 16
                    g1 = min(nf, g0 + 16)
                    w = (g1 - g0) * H
                    nc.tensor.matmul(pss[:, :w], ones_col[:],
                                     z_tile[:, g0:g1, :],
                                     start=(b == 0), stop=(b == nblk - 1))
                for b in range(nblk):
                    g0 = b * 16
                    g1 = min(nf, g0 + 16)
                    w = (g1 - g0) * H
                    nc.tensor.matmul(psq[:, :w], ones_col[:],
                                     zsq_tile[:, g0:g1, :],
                                     start=(b == 0), stop=(b == nblk - 1))
                stp = smp.tile([1, 2 * H], f32, tag=f"stp{li}")
                ncnt = min(16, nf)
                import concourse.bass as bass2
                nc.vector.tensor_reduce(
                    stp[:, 0:H],
                    bass2.AP(pss.tensor, pss[:].offset,
                             [pss[:].ap[0], [1, H], [H, ncnt]]),
                    axis=mybir.AxisListType.X, op=ADD)
                nc.vector.tensor_reduce(
                    stp[:, H:2 * H],
                    bass2.AP(psq.tensor, psq[:].offset,
                             [psq[:].ap[0], [1, H], [H, ncnt]]),
                    axis=mybir.AxisListType.X, op=ADD)
                nc.sync.dma_start(stb_in[li].ap(), stp[:])
                nc.gpsimd.collective_compute(
                    "AllReduce", ADD, replica_groups=groups,
                    ins=[stb_in[li].ap()], outs=[stb_out[li].ap()])
                str_ = smp.tile([1, 2 * H], f32, tag=f"str{li}")
                nc.sync.dma_start(str_[:], stb_out[li].ap())
                return str_

            def emit_affine(str_, gamma, beta, li):
                mean = smp.tile([1, H], f32, tag=f"mean{li}")
                nc.vector.tensor_scalar_mul(mean[:], str_[:, 0:H],
                                            1.0 / n_real)
                var = smp.tile([1, H], f32, tag=f"var{li}")
                nc.vector.tensor_scalar_mul(var[:], str_[:, H:2 * H],
                                            1.0 / n_real)
                msq = smp.tile([1, H], f32, tag=f"msq{li}")
                nc.vector.tensor_tensor(msq[:], mean[:], mean[:], op=MULT)
                nc.vector.tensor_tensor(var[:], var[:], msq[:], op=SUB)
                sd = smp.tile([1, H], f32, tag=f"sd{li}")
                nc.scalar.activation(sd[:], var[:],
                                     mybir.ActivationFunctionType.Sqrt,
                                     bias=EPS)
                inv = smp.tile([1, H], f32, tag=f"inv{li}")
                nc.vector.reciprocal(inv[:], sd[:])
                scl = smp.tile([1, H], f32, tag=f"scl{li}")
                nc.vector.tensor_tensor(scl[:], gamma, inv[:], op=MULT)
                shf = smp.tile([1, H], f32, tag=f"shf{li}")
                nc.vector.tensor_tensor(shf[:], mean[:], scl[:], op=MULT)
                nc.vector.tensor_tensor(shf[:], beta, shf[:], op=SUB)
                return scl, shf

            def emit_bn_relu(z_tile, scl, shf, out_tile):
                s128 = pe_broadcast_row(scl[:], H)
                f128 = pe_broadcast_row(shf[:], H)
                nc.vector.tensor_tensor(out_tile[:], z_tile[:],
                                        row_bc(s128[:]), op=MULT)
                nc.vector.tensor_tensor(out_tile[:], out_tile[:],
                                        row_bc(f128[:]), op=ADD)
                nc.scalar.activation(out_tile[:], out_tile[:],
                                     mybir.ActivationFunctionType.Relu)

            def emit_table(h_tile, w_ap, tall_h):
                """stag[:,:,0:H] = (h*nm) @ W; DMA shard; AllGather."""
                nc.vector.tensor_tensor(h_tile[:], h_tile[:],
                                        bc_feat(nm[:]), op=MULT)
                for g in range(nf):
                    tp = psp.tile([H, 128], f32, tag="ps_tp")
                    nc.tensor.transpose(tp[:], h_tile[:, g, :], ident[:])
                    hts = bcp.tile([H, 128], f32, tag="sb_ht")
                    nc.vector.tensor_copy(hts[:], tp[:])
                    tm = psp.tile([128, H], f32, tag="ps_tm")
                    nc.tensor.matmul(tm[:], hts[:], w_ap, start=True,
                                     stop=True)
                    nc.vector.tensor_copy(stag[:, g, 0:H], tm[:])
                nc.sync.dma_start(
                    tsh.ap().rearrange("(p g) f -> p (g f)", p=128),
                    stag[:].rearrange("p g f -> p (g f)"))
                nc.gpsimd.collective_compute(
                    "AllGather", mybir.AluOpType.bypass,
                    replica_groups=groups,
                    ins=[tsh.ap()], outs=[tall_h.ap()])

            # small zero tile; aggD zeroed via broadcast DMA
            zero64 = smp.tile([128, EL], f32, tag="zero64")
            nc.gpsimd.memset(zero64[:], 0.0)

            # ================= layer 1 =================
            # table1 rows: [xhat, 0, 0, ...] (scalar in col 0)
            xhat = smp.tile([128, nf], f32, tag="xhat")
            nc.vector.tensor_tensor(xhat[:], xs[:], nm[:], op=MULT)
            nc.vector.tensor_copy(stag[:, :, 0:1],
                                  ap_append(xhat[:], [[0, 1]]))
            nc.sync.dma_start(
                tsh.ap().rearrange("(p g) f -> p (g f)", p=128),
                stag[:].rearrange("p g f -> p (g f)"))
            nc.gpsimd.collective_compute(
                "AllGather", mybir.AluOpType.bypass, replica_groups=groups,
                ins=[tsh.ap()], outs=[tall[0].ap()])

            emit_aggregate(tall[0], aggD[0], 0, 1)   # agg1 -> zT[:,:,0]

            # s1 = norm * (agg1 + xhat)
            aggs = smp.tile([128, nf], f32, tag="aggs")
            nc.vector.tensor_copy(aggs[:], zT[:, :, 0])
            nc.vector.tensor_tensor(aggs[:], aggs[:], xhat[:], op=ADD)
            nc.vector.tensor_tensor(aggs[:], aggs[:], norm[:], op=MULT)

            # layer-1 stats from s1
            s1sq = smp.tile([128, nf], f32, tag="s1sq")
            nc.vector.tensor_tensor(s1sq[:], aggs[:], aggs[:], op=MULT)
            ps1 = pstat.tile([1, 16 * H], f32, tag="pss")
            nc.tensor.matmul(ps1[:, :nf], ones_col[:], aggs[:],
                             start=True, stop=True)
            ps1b = pstat.tile([1, 16 * H], f32, tag="psq")
            nc.tensor.matmul(ps1b[:, :nf], ones_col[:], s1sq[:],
                             start=True, stop=True)
            s1pack = smp.tile([1, 2 * H], f32, tag="s1pack")
            nc.gpsimd.memset(s1pack[:], 0.0)
            nc.vector.tensor_reduce(s1pack[:, 0:1], ps1[:, :nf],
                                    axis=mybir.AxisListType.X, op=ADD)
            nc.vector.tensor_reduce(s1pack[:, 1:2], ps1b[:, :nf],
                                    axis=mybir.AxisListType.X, op=ADD)
            nc.sync.dma_start(stb_in[0].ap(), s1pack[:])
            nc.gpsimd.collective_compute(
                "AllReduce", ADD, replica_groups=groups,
                ins=[stb_in[0].ap()], outs=[stb_out[0].ap()])
            s1r = smp.tile([1, 2 * H], f32, tag="s1r")
            nc.sync.dma_start(s1r[:], stb_out[0].ap())

            import concourse.bass as bassm
            sn = smp.tile([1, 1], f32, tag="sn")
            nc.vector.tensor_scalar_mul(sn[:], s1r[:, 0:1], 1.0 / n_real)
            qn1 = smp.tile([1, 1], f32, tag="qn1")
            nc.vector.tensor_scalar_mul(qn1[:], s1r[:, 1:2], 1.0 / n_real)
            cvar = smp.tile([1, 1], f32, tag="cvar")
            nc.vector.tensor_tensor(cvar[:], sn[:], sn[:], op=MULT)
            nc.vector.tensor_tensor(cvar[:], qn1[:], cvar[:], op=SUB)
            w1sq = smp.tile([1, H], f32, tag="w1sq")
            nc.vector.tensor_tensor(w1sq[:], wt["w1"][:], wt["w1"][:],
                                    op=MULT)
            mean1 = smp.tile([1, H], f32, tag="mean1")
            nc.vector.tensor_tensor(
                mean1[:], wt["w1"][:],
                bassm.AP(sn.tensor, sn[:].offset, [sn[:].ap[0], [0, H]]),
                op=MULT)
            var1 = smp.tile([1, H], f32, tag="var1")
            nc.vector.tensor_tensor(
                var1[:], w1sq[:],
                bassm.AP(cvar.tensor, cvar[:].offset, [cvar[:].ap[0], [0, H]]),
                op=MULT)
            sd1 = smp.tile([1, H], f32, tag="sd1")
            nc.scalar.activation(sd1[:], var1[:],
                                 mybir.ActivationFunctionType.Sqrt, bias=EPS)
            inv1 = smp.tile([1, H], f32, tag="inv1")
            nc.vector.reciprocal(inv1[:], sd1[:])
            scl1 = smp.tile([1, H], f32, tag="scl1")
            nc.vector.tensor_tensor(scl1[:], wt["g1"][:], inv1[:], op=MULT)
            shf1 = smp.tile([1, H], f32, tag="shf1")
            nc.vector.tensor_tensor(shf1[:], mean1[:], scl1[:], op=MULT)
            nc.vector.tensor_tensor(shf1[:], wt["be1"][:], shf1[:], op=SUB)
            w1s = smp.tile([1, H], f32, tag="w1s")
            nc.vector.tensor_tensor(w1s[:], wt["w1"][:], scl1[:], op=MULT)

            w1s128 = pe_broadcast_row(w1s[:], H)
            shf1128 = pe_broadcast_row(shf1[:], H)
            nc.vector.tensor_tensor(hT[:], bc_feat(aggs[:]),
                                    row_bc(w1s128[:]), op=MULT)
            nc.vector.tensor_tensor(hT[:], hT[:], row_bc(shf1128[:]), op=ADD)
            nc.scalar.activation(hT[:], hT[:],
                                 mybir.ActivationFunctionType.Relu)

            # ================= layers 2, 3 =================
            for li, (wname, gname, bname) in enumerate(
                    [("w2", "g2", "be2"), ("w3", "g3", "be3")], start=1):
                emit_table(hT, wt[wname][:], tall[li])
                emit_aggregate(tall[li], aggD[li], li, H)  # -> zT
                nc.vector.tensor_tensor(zT[:], zT[:], stag[:, :, 0:H],
                                        op=ADD)
                nc.vector.tensor_tensor(zT[:], zT[:], bc_feat(norm[:]),
                                        op=MULT)
                str_ = emit_stats(zT, hT, li)
                scl, shf = emit_affine(str_, wt[gname][:], wt[bname][:], li)
                emit_bn_relu(zT, scl, shf, hT)

            # ================= final fc =================
            fcw128 = pe_broadcast_row(wt["fcw"][:], H)
            fcb128 = pe_broadcast_row(wt["fcb"][:], 1)
            ytmp = sb.tile([128, nf, H], f32, tag="z", name="ytmp")
            nc.vector.tensor_tensor(ytmp[:], hT[:], row_bc(fcw128[:]),
                                    op=MULT)
            yt = smp.tile([128, nf], f32, tag="yt")
            nc.vector.tensor_reduce(yt[:], ytmp[:],
                                    axis=mybir.AxisListType.X, op=ADD)
            nc.vector.tensor_scalar_add(yt[:], yt[:], fcb128[:, 0:1])
            nc.sync.dma_start(yout, yt[:])

    nc.compile()
    return nc


# ---------------------------------------------------------------------------
# Entry point
# ---------------------------------------------------------------------------

def _make_in_maps(plan, weights):
    ins = []
    for c in range(plan["nc_cores"]):
        m = {
            "xin": np.ascontiguousarray(plan["x_t"][c], np.float32),
            "degin": np.ascontiguousarray(plan["deg_t"][c], np.float32),
            "maskin": np.ascontiguousarray(plan["mask_t"][c], np.float32),
            "gidxin": np.ascontiguousarray(plan["gidx_w"][c], np.int16),
            "sidxin": np.ascontiguousarray(plan["sidx_w"][c], np.int16),
        }
        m.update({k: np.ascontiguousarray(v, np.float32)
                  for k, v in weights.items()})
        ins.append(m)
    return ins


def _extract_weights(inputs):
    w = {
        "w1": np.asarray(inputs["W1"], np.float32).reshape(1, H),
        "w2": np.asarray(inputs["W2"], np.float32),
        "w3": np.asarray(inputs["W3"], np.float32),
        "fcw": np.asarray(inputs["fcW"], np.float32).reshape(1, H),
        "fcb": np.asarray(inputs["fcb"], np.float32).reshape(1, 1),
    }
    for i in (1, 2, 3):
        w[f"g{i}"] = np.asarray(inputs[f"g{i}"], np.float32).reshape(1, H)
        w[f"be{i}"] = np.asarray(inputs[f"be{i}"], np.float32).reshape(1, H)
    return w


def _unshard(plan, results):
    n = plan["n_nodes"]
    y = np.zeros((n, 1), np.float32)
    r = plan["rank_of"]
    c = plan["core_of"]
    ys = np.stack([np.asarray(results[i]["yout"])
                   for i in range(plan["nc_cores"])])
    y[:, 0] = ys[c, r % 128, r // 128]
    return y


_CACHE = {}


def kernel(**inputs):
    edge_index = np.asarray(inputs["edge_index"])
    src = edge_index[0].astype(np.int64)
    dst = edge_index[1].astype(np.int64)
    x = np.asarray(inputs["x"], np.float32)

    if "prog" not in _CACHE:
        plan = build_plan(src, dst, x)
        nc = build_program(plan)
        _CACHE["prog"] = (plan, nc)
    plan, nc = _CACHE["prog"]
    weights = _extract_weights(inputs)
    in_maps = _make_in_maps(plan, weights)
    _CACHE["in_maps"] = in_maps

    from concourse import bass_utils
    res = bass_utils.run_bass_kernel_spmd(
        nc, in_maps, core_ids=list(range(plan["nc_cores"])), trace=False)
    return _unshard(plan, res.results)


def timed_run(iters=5):
    """Persistent-executable timing; call kernel() first."""
    import time
    import jax
    from jax.sharding import Mesh, PartitionSpec
    from jax.experimental.shard_map import shard_map
    import concourse.mybir as mybir
    from concourse import bass2jax

    plan, nc = _CACHE["prog"]
    in_maps = _CACHE["in_maps"]
    n_cores = plan["nc_cores"]

    bass2jax.install_neuronx_cc_hook()
    in_names, out_names, out_avals, zero_outs = [], [], [], []
    partition_name = (nc.partition_id_tensor.name
                      if nc.partition_id_tensor else None)
    for alloc in nc.m.functions[0].allocations:
        if not isinstance(alloc, mybir.MemoryLocationSet):
            continue
        name = alloc.memorylocations[0].name
        if alloc.kind == "ExternalInput":
            if name != partition_name:
                in_names.append(name)
        elif alloc.kind == "ExternalOutput":
            out_names.append(name)
            shape = tuple(alloc.tensor_shape)
            dtype = mybir.dt.np(alloc.dtype)
            out_avals.append(jax.core.ShapedArray(shape, dtype))
            zero_outs.append(np.zeros(shape, dtype))
    n_params = len(in_names)
    all_names = list(in_names) + out_names
    if partition_name is not None:
        all_names.append(partition_name)

    def _body(*args):
        operands = list(args)
        if partition_name is not None:
            operands.append(bass2jax.partition_id_tensor())
        return tuple(bass2jax._bass_exec_p.bind(
            *operands, out_avals=tuple(out_avals), in_names=tuple(all_names),
            out_names=tuple(out_names), lowering_input_output_aliases=(),
            sim_require_finite=True, sim_require_nnan=True, nc=nc))

    devices = jax.devices()[:n_cores]
    mesh = Mesh(np.asarray(devices), ("core",))
    n_outs = len(out_names)
    donate = tuple(range(n_params, n_params + n_outs))
    sharded = jax.jit(
        shard_map(_body, mesh=mesh,
                  in_specs=(PartitionSpec("core"),) * (n_params + n_outs),
                  out_specs=(PartitionSpec("core"),) * n_outs,
                  check_rep=False),
        donate_argnums=donate, keep_unused=True)
    concat_in = [
        np.concatenate([np.asarray(in_maps[c][nm]) for c in range(n_cores)],
                       axis=0)
        for nm in in_names]
    in_args = [jax.device_put(a) for a in concat_in]
    times = []
    out = None
    for i in range(iters + 1):
        zargs = [jax.device_put(
            np.zeros((n_cores * z.shape[0], *z.shape[1:]), z.dtype))
            for z in zero_outs]
        jax.block_until_ready(zargs)
        t0 = time.perf_counter()
        out = sharded(*in_args, *zargs)
        jax.block_until_ready(out)
        if i > 0:
            times.append(time.perf_counter() - t0)
    outs_np = [np.asarray(o) for o in out]
    results = [
        {nm: outs_np[i].reshape(n_cores, *out_avals[i].shape)[c]
         for i, nm in enumerate(out_names)}
        for c in range(n_cores)]
    return times, _unshard(plan, results)

